# revision 31
# baseline (speedup 1.0000x reference)
"""Trainium2 Bass kernel for nn_BboxRegressionLoss (topk_masking).

Math notes
----------
reference computes, with iou1ds = iou2ds reshaped [M, P] (mask2d all-ones):
    mask = scatter(top3_idx) | (iou1ds > 0.5)
    loss = |so + starts - tgt_s| + |eo + ends - tgt_e|     (per [M, P] element)
    out  = (loss * mask).sum() / mask.sum()

Key identity: if a row has >= TOPK elements with iou > 0.5, its top-TOPK
elements are all already inside the threshold mask, so mask == (iou > 0.5)
EXACTLY for that row. We compute per-row counts of (iou > 0.5) on device
anyway (needed for mask.sum()), so we can verify the identity for every row
after the fact and fall back to a full numpy replica in the (practically
impossible for uniform iou) case where some row has fewer than TOPK
above-threshold elements.

Device layout (per core, M_loc = 128 targets on partitions, P chunked):
    PE     : replicate K source-offset rows -> 128 target partitions via a
             0/1 matmul (avoids re-reading so/eo 4x from HBM)
    ACT    : a = Abs(so2rep - tgt_s), b = Abs(eo2rep - tgt_e)   (bias fusion)
    DVE    : scalar_tensor_tensor (iou > 0.5) * a  with fused row-sum accum
             (and same for b); tensor_scalar (iou > 0.5) accum for the count
Host folds the `starts`/`ends` proposal-grid constants into so/eo (so2/eo2),
sums the 8 x [128, 2] partials in f64 and divides.

bf16 is used for iou/so2/eo2/a/b (DVE 16-bit 2x mode + half the DMA bytes);
accumulation stays f32. Measured end-to-end rel err vs the f32 reference is
~1e-4, far inside the tolerance.
"""

import os

import numpy as np

TOPK = 3
IOU_THRESHOLD = 0.5
N_CORES = 8

# filled by kernel() on every call; test.py reads these
LAST_EXEC_TIME_NS = None
LAST_RESULTS = None

_NC_CACHE = {}

_AXON_PJRT_SO = "/opt/axon/libaxon_pjrt.so"


def _ensure_ntff_hook():
    """concourse.bass_utils hard-imports antenv.axon_hooks when tracing is
    requested (BASS_TRACE=1). Some images lack that module; provide a shim
    wired to libaxon_pjrt.so's NRT profile entry points so tracing works
    (and a missing hook degrades to an untraced run instead of crashing)."""
    try:
        from antenv.axon_hooks import get_axon_ntff_profile_hook  # noqa: F401

        return
    except ImportError:
        pass

    import contextlib
    import ctypes
    import sys
    import types

    mod = types.ModuleType("antenv.axon_hooks")
    state = {"hook": None}
    mod.set_axon_ntff_profile_hook = lambda h: state.__setitem__("hook", h)
    mod.get_axon_ntff_profile_hook = lambda: state["hook"]
    sys.modules["antenv.axon_hooks"] = mod
    try:
        import antenv

        antenv.axon_hooks = mod
    except ImportError:
        pass

    if not os.path.exists(_AXON_PJRT_SO):
        return
    lib = ctypes.CDLL(_AXON_PJRT_SO)
    if not hasattr(lib, "axon_start_nrt_profile"):
        return
    lib.axon_start_nrt_profile.argtypes = [
        ctypes.POINTER(ctypes.c_int64),
        ctypes.c_size_t,
    ]
    lib.axon_start_nrt_profile.restype = ctypes.c_int64
    lib.axon_stop_nrt_profile.argtypes = [ctypes.c_char_p]
    lib.axon_stop_nrt_profile.restype = ctypes.c_int64

    @contextlib.contextmanager
    def _hook(output_dir, device_ids):
        import jax

        jax.devices()
        if device_ids:
            ids = (ctypes.c_int64 * len(device_ids))(*device_ids)
            rc = lib.axon_start_nrt_profile(ids, len(device_ids))
        else:
            rc = lib.axon_start_nrt_profile(None, 0)
        if rc != 0:
            raise RuntimeError(f"axon_start_nrt_profile rc={rc}")
        try:
            yield
        finally:
            n = lib.axon_stop_nrt_profile(str(output_dir).encode())
            if n < 0:
                raise RuntimeError(f"axon_stop_nrt_profile rc={n}")

    mod.set_axon_ntff_profile_hook(_hook)


def _build_nc(K, M_loc, P, C):
    import concourse.bacc as bacc
    import concourse.mybir as mybir
    from concourse.tile import TileContext

    f32 = mybir.dt.float32
    bf16 = mybir.dt.bfloat16
    NCH = P // C
    assert P % C == 0 and C % 512 == 0
    MMW = C // 512  # matmuls per chunk per tensor (PSUM bank = 512 f32)

    nc = bacc.Bacc()
    iou = nc.declare_dram_parameter("iou", [M_loc, P], bf16, isOutput=False)
    so2 = nc.declare_dram_parameter("so2", [K, P], bf16, isOutput=False)
    eo2 = nc.declare_dram_parameter("eo2", [K, P], bf16, isOutput=False)
    repl = nc.declare_dram_parameter("repl", [K, M_loc], bf16, isOutput=False)
    ntgt = nc.declare_dram_parameter("ntgt", [M_loc, 2], f32, isOutput=False)
    out = nc.declare_dram_parameter("out", [M_loc, 2], f32, isOutput=True)

    with TileContext(nc) as tc:
        with (
            tc.tile_pool(name="singles", bufs=1) as singles,
            # one slot per chunk: iou DMAs are all emitted up-front, so slots
            # must never be recycled (recycling would need WAR deps on readers
            # that don't exist yet at emission time)
            tc.tile_pool(name="io", bufs=P // C) as io,
            tc.tile_pool(name="work", bufs=3) as work,
            # [128, C] f32 = C/512 banks per tile; ps_s/ps_e ping-pong fills
            # all 8 PSUM banks at C=2048
            tc.tile_pool(name="psum", bufs=1, space="PSUM") as psum,
        ):
            R_sb = singles.tile([K, M_loc], bf16)
            nc.sync.dma_start(out=R_sb, in_=repl[:, :])
            ntgt_sb = singles.tile([M_loc, 2], f32)
            nc.sync.dma_start(out=ntgt_sb, in_=ntgt[:, :])
            # source-offset rows stay resident (bf16 [K, P] = K partitions x 32KB).
            # Loaded as one tile PER CHUNK-GROUP so early matmuls don't wait on
            # the whole 1MB transfer (Tile deps are per-tile). DMA emission
            # order: piece 0 + the first iou chunks FIRST so the pipeline
            # fills immediately, remaining pieces next, rest of iou after.
            so_piece = C
            so2_sbs, eo2_sbs, iou_tiles = [], [], []

            def load_piece(pi):
                psl = slice(pi * so_piece, (pi + 1) * so_piece)
                s_t = singles.tile([K, so_piece], bf16, tag=f"so2_sb{pi}")
                nc.sync.dma_start(out=s_t, in_=so2[:, psl])
                so2_sbs.append(s_t)
                e_t = singles.tile([K, so_piece], bf16, tag=f"eo2_sb{pi}")
                nc.sync.dma_start(out=e_t, in_=eo2[:, psl])
                eo2_sbs.append(e_t)

            def load_iou(ci):
                sl = slice(ci * C, (ci + 1) * C)
                t = io.tile([M_loc, C], bf16, tag="iouc")
                nc.sync.dma_start(out=t, in_=iou[:, sl])
                iou_tiles.append(t)

            # interleave so the first chunk's operands land first
            for ci in range(NCH):
                load_piece(ci)
                load_iou(ci)

            accL = singles.tile([M_loc, 2 * NCH], f32)
            NCH_DVE = max(0, min(NCH, (6 * NCH) // 16))  # ~3 of 8 at C=2048  # count chunks on DVE
            NCH_ACT = NCH - NCH_DVE                      # count chunks on ACT (Sign)
            accM = singles.tile([M_loc, max(NCH_DVE, 1)], f32)
            accS = singles.tile([M_loc, max(NCH_ACT, 1)], f32)
            neg_half = singles.tile([M_loc, 1], f32)
            nc.vector.memset(neg_half, -IOU_THRESHOLD)

            for ci in range(NCH):
                iouc = iou_tiles[ci]

                so2rep = psum.tile([M_loc, C], f32, tag="ps_s")
                eo2rep = psum.tile([M_loc, C], f32, tag="ps_e")
                for mi in range(MMW):
                    psl = slice(mi * 512, (mi + 1) * 512)
                    nc.tensor.matmul(
                        so2rep[:, psl], lhsT=R_sb,
                        rhs=so2_sbs[ci][:, psl],
                        start=True, stop=True,
                    )
                for mi in range(MMW):
                    psl = slice(mi * 512, (mi + 1) * 512)
                    nc.tensor.matmul(
                        eo2rep[:, psl], lhsT=R_sb,
                        rhs=eo2_sbs[ci][:, psl],
                        start=True, stop=True,
                    )

                a = work.tile([M_loc, C], bf16, tag="a")
                nc.scalar.activation(
                    out=a,
                    in_=so2rep,
                    func=mybir.ActivationFunctionType.Abs,
                    bias=ntgt_sb[:, 0:1],
                    scale=1.0,
                )
                b = work.tile([M_loc, C], bf16, tag="b")
                nc.scalar.activation(
                    out=b,
                    in_=eo2rep,
                    func=mybir.ActivationFunctionType.Abs,
                    bias=ntgt_sb[:, 1:2],
                    scale=1.0,
                )

                # NOTE: offloading an op to GPSIMD is a net loss here - GpSimd
                # and DVE share SBUF ports (exclusive lock) and both engines
                # drop to half rate when streaming concurrently.
                junk_a = work.tile([M_loc, C], bf16, tag="junk_a")
                nc.vector.scalar_tensor_tensor(
                    out=junk_a,
                    in0=iouc,
                    scalar=IOU_THRESHOLD,
                    in1=a,
                    op0=mybir.AluOpType.is_gt,
                    op1=mybir.AluOpType.mult,
                    accum_out=accL[:, ci : ci + 1],
                )
                junk_b = work.tile([M_loc, C], bf16, tag="junk_b")
                nc.vector.scalar_tensor_tensor(
                    out=junk_b,
                    in0=iouc,
                    scalar=IOU_THRESHOLD,
                    in1=b,
                    op0=mybir.AluOpType.is_gt,
                    op1=mybir.AluOpType.mult,
                    accum_out=accL[:, NCH + ci : NCH + ci + 1],
                )
                if ci < NCH_DVE:
                    # mask count on DVE (accum_out reduce op is op1)
                    junk_m = work.tile([M_loc, C], bf16, tag="junk_m")
                    nc.vector.tensor_scalar(
                        out=junk_m,
                        in0=iouc,
                        scalar1=IOU_THRESHOLD,
                        scalar2=None,
                        op0=mybir.AluOpType.is_gt,
                        op1=mybir.AluOpType.add,
                        accum_out=accM[:, ci : ci + 1],
                    )
                else:
                    # mask count on ACT: accum of Sign(iou-0.5). The host
                    # nudges bf16 iou off the exact 0.5 value in both
                    # directions, so sign is strictly +-1 and
                    # count = (accum + C) / 2 exactly.
                    junk_s = work.tile([M_loc, C], bf16, tag="junk_s")
                    nc.scalar.activation(
                        out=junk_s,
                        in_=iouc,
                        func=mybir.ActivationFunctionType.Sign,
                        bias=neg_half[:, 0:1],
                        scale=1.0,
                        accum_out=accS[:, ci - NCH_DVE : ci - NCH_DVE + 1],
                    )

            outsb = singles.tile([M_loc, 2], f32)
            nc.vector.reduce_sum(
                out=outsb[:, 0:1], in_=accL, axis=mybir.AxisListType.X
            )
            # count = sum(accM) + (sum(accS) + NCH_ACT*C)/2
            cnt_m = singles.tile([M_loc, 1], f32)
            if NCH_DVE > 0:
                nc.vector.reduce_sum(out=cnt_m, in_=accM, axis=mybir.AxisListType.X)
            else:
                nc.vector.memset(cnt_m, 0.0)
            cnt_s = singles.tile([M_loc, 1], f32)
            if NCH_ACT > 0:
                nc.vector.reduce_sum(out=cnt_s, in_=accS, axis=mybir.AxisListType.X)
            else:
                nc.vector.memset(cnt_s, 0.0)
            cnt_s2 = singles.tile([M_loc, 1], f32)
            nc.vector.tensor_scalar(
                out=cnt_s2,
                in0=cnt_s,
                scalar1=0.5,
                scalar2=float(NCH_ACT * C) / 2.0,
                op0=mybir.AluOpType.mult,
                op1=mybir.AluOpType.add,
            )
            nc.vector.tensor_tensor(
                out=outsb[:, 1:2], in0=cnt_m, in1=cnt_s2,
                op=mybir.AluOpType.add,
            )
            nc.sync.dma_start(out=out[:, :], in_=outsb)

    nc.compile()
    return nc


def _scatter_m2s(num_targets, S, M):
    """target index -> source video index, mirroring jnp.repeat(
    arange(S), num_targets, total_repeat_length=M)."""
    cum = np.cumsum(num_targets.astype(np.int64))
    idx = np.searchsorted(cum, np.arange(M), side="right")
    return np.clip(idx, 0, S - 1).astype(np.int64)


def _numpy_reference(start_offset, end_offset, tgt_moments, num_targets, iou2ds, mask2d):
    """Exact numpy replica of reference.py (topk fallback path)."""
    M, N, _ = iou2ds.shape
    S, P = start_offset.shape
    scatter = _scatter_m2s(num_targets, S, M)
    so = start_offset[scatter]
    eo = end_offset[scatter]
    r, c = np.nonzero(mask2d)
    if r.shape[0] < P:
        pad = P - r.shape[0]
        r = np.concatenate([r, np.zeros(pad, dtype=r.dtype)])
        c = np.concatenate([c, np.zeros(pad, dtype=c.dtype)])
    else:
        r, c = r[:P], c[:P]
    iou1 = iou2ds.reshape(M, N * N)[:, r * N + c]
    # top-k scatter mask + threshold mask
    topk_idx = np.argsort(-iou1, axis=1, kind="stable")[:, :TOPK]
    mask = np.zeros((M, P), dtype=np.float32)
    np.put_along_axis(mask, topk_idx, 1.0, axis=1)
    mask = np.where(iou1 > IOU_THRESHOLD, np.float32(1.0), mask)
    starts = (r.astype(np.float32) / N)[None, :]
    ends = ((c.astype(np.float32) + 1.0) / N)[None, :]
    sot = tgt_moments[:, 0:1] - starts
    eot = tgt_moments[:, 1:2] - ends
    loss = np.abs(so - sot) + np.abs(eo - eot)
    return np.float32((loss * mask).sum(dtype=np.float64) / mask.sum(dtype=np.float64))


def kernel(**inputs):
    global LAST_EXEC_TIME_NS, LAST_RESULTS
    _ensure_ntff_hook()
    import ml_dtypes

    from concourse.bass_utils import run_bass_kernel_spmd

    start_offset = np.asarray(inputs["start_offset"], dtype=np.float32)
    end_offset = np.asarray(inputs["end_offset"], dtype=np.float32)
    tgt_moments = np.asarray(inputs["tgt_moments"], dtype=np.float32)
    num_targets = np.asarray(inputs["num_targets"])
    iou2ds = np.asarray(inputs["iou2ds"], dtype=np.float32)
    mask2d = np.asarray(inputs["mask2d"])

    bf16 = ml_dtypes.bfloat16

    M, N, _ = iou2ds.shape
    S, P = start_offset.shape
    assert M % N_CORES == 0
    M_loc = M // N_CORES

    # proposal-grid constants from mask2d (row-major nonzero, padded like jnp)
    r, c = np.nonzero(mask2d)
    if r.shape[0] < P:
        pad = P - r.shape[0]
        r = np.concatenate([r, np.zeros(pad, dtype=r.dtype)])
        c = np.concatenate([c, np.zeros(pad, dtype=c.dtype)])
    else:
        r, c = r[:P], c[:P]
    starts = r.astype(np.float32) / np.float32(N)
    ends = (c.astype(np.float32) + np.float32(1.0)) / np.float32(N)

    # iou1ds = iou2ds[:, r, c]; identity reshape when mask2d is all ones
    flat_idx = r.astype(np.int64) * N + c.astype(np.int64)
    iou_flat = iou2ds.reshape(M, N * N)
    if not (flat_idx == np.arange(P)).all():
        iou_flat = np.ascontiguousarray(iou_flat[:, flat_idx])
    # bf16 halves the iou DMA bytes, but values that round exactly onto the
    # 0.5 threshold would corrupt the comparison. Nudge those one bf16 ulp
    # away from 0.5 in the direction of their f32 value; this makes
    # (iou_bf16 > 0.5) == (iou_f32 > 0.5) for every element AND leaves no
    # element exactly at 0.5, so the device's Sign(iou-0.5) count path is
    # strictly +-1 (exact counts).
    iou_bf16 = iou_flat.astype(bf16)
    on_thr = iou_bf16 == bf16(IOU_THRESHOLD)
    above = on_thr & (iou_flat > np.float32(IOU_THRESHOLD))
    below = on_thr & ~above
    if above.any():
        iou_bf16[above] = bf16(0.50390625)  # nextafter(0.5, up) in bf16
    if below.any():
        iou_bf16[below] = bf16(0.498046875)  # nextafter(0.5, down) in bf16

    # fold grid constants into the offsets: loss_a = |so2 - tgt_s|
    so2_full = (start_offset + starts[None, :]).astype(bf16)
    eo2_full = (end_offset + ends[None, :]).astype(bf16)

    # per-core source-row windows + replication matrices
    scatter = _scatter_m2s(num_targets, S, M)
    src_lo = np.empty(N_CORES, dtype=np.int64)
    n_src = np.empty(N_CORES, dtype=np.int64)
    for core in range(N_CORES):
        seg = scatter[core * M_loc : (core + 1) * M_loc]
        src_lo[core] = seg[0]
        n_src[core] = seg[-1] - seg[0] + 1
    K = int(n_src.max())

    in_maps = []
    for core in range(N_CORES):
        seg = scatter[core * M_loc : (core + 1) * M_loc]
        lo = int(src_lo[core])
        so2_c = np.zeros((K, P), dtype=bf16)
        eo2_c = np.zeros((K, P), dtype=bf16)
        hi = min(lo + K, S)
        so2_c[: hi - lo] = so2_full[lo:hi]
        eo2_c[: hi - lo] = eo2_full[lo:hi]
        repl = np.zeros((K, M_loc), dtype=bf16)
        repl[seg - lo, np.arange(M_loc)] = 1.0
        ntgt = np.ascontiguousarray(
            -tgt_moments[core * M_loc : (core + 1) * M_loc, :]
        ).astype(np.float32)
        in_maps.append(
            {
                "iou": np.ascontiguousarray(iou_bf16[core * M_loc : (core + 1) * M_loc]),
                "so2": so2_c,
                "eo2": eo2_c,
                "repl": repl,
                "ntgt": ntgt,
            }
        )

    cache_key = (K, M_loc, P)
    if cache_key not in _NC_CACHE:
        _NC_CACHE[cache_key] = _build_nc(K, M_loc, P, C=2048)
    nc = _NC_CACHE[cache_key]

    res = run_bass_kernel_spmd(nc, in_maps, list(range(N_CORES)))
    LAST_EXEC_TIME_NS = res.exec_time_ns
    LAST_RESULTS = res

    loss_sum = 0.0
    mask_sum = 0.0
    min_count = np.inf
    for core in range(N_CORES):
        part = res.results[core]["out"]  # [M_loc, 2]
        loss_sum += part[:, 0].sum(dtype=np.float64)
        mask_sum += part[:, 1].sum(dtype=np.float64)
        min_count = min(min_count, part[:, 1].min())

    if min_count < TOPK:
        # some row's top-k reaches below the threshold: the threshold mask is
        # not exact there -> use the exact (slow) host path
        return _numpy_reference(
            start_offset, end_offset, tgt_moments, num_targets, iou2ds, mask2d
        )

    return np.float32(loss_sum / mask_sum)


# revision 33
# speedup vs baseline: 1.0473x; 1.0473x over previous
"""Trainium2 Bass kernel for nn_BboxRegressionLoss (topk_masking).

Math notes
----------
reference computes, with iou1ds = iou2ds reshaped [M, P] (mask2d all-ones):
    mask = scatter(top3_idx) | (iou1ds > 0.5)
    loss = |so + starts - tgt_s| + |eo + ends - tgt_e|     (per [M, P] element)
    out  = (loss * mask).sum() / mask.sum()

Key identity: if a row has >= TOPK elements with iou > 0.5, its top-TOPK
elements are all already inside the threshold mask, so mask == (iou > 0.5)
EXACTLY for that row. We compute per-row counts of (iou > 0.5) on device
anyway (needed for mask.sum()), so we can verify the identity for every row
after the fact and fall back to a full numpy replica in the (practically
impossible for uniform iou) case where some row has fewer than TOPK
above-threshold elements.

Device layout (per core, M_loc = 128 targets on partitions, P chunked):
    PE     : replicate K source-offset rows -> 128 target partitions via a
             0/1 matmul (avoids re-reading so/eo 4x from HBM)
    ACT    : a = Abs(so2rep - tgt_s), b = Abs(eo2rep - tgt_e)   (bias fusion)
    DVE    : scalar_tensor_tensor (iou > 0.5) * a  with fused row-sum accum
             (and same for b); tensor_scalar (iou > 0.5) accum for the count
Host folds the `starts`/`ends` proposal-grid constants into so/eo (so2/eo2),
sums the 8 x [128, 2] partials in f64 and divides.

bf16 is used for iou/so2/eo2/a/b (DVE 16-bit 2x mode + half the DMA bytes);
accumulation stays f32. Measured end-to-end rel err vs the f32 reference is
~1e-4, far inside the tolerance.
"""

import os

import numpy as np

TOPK = 3
IOU_THRESHOLD = 0.5
N_CORES = 8

# filled by kernel() on every call; test.py reads these
LAST_EXEC_TIME_NS = None
LAST_RESULTS = None

_NC_CACHE = {}

_AXON_PJRT_SO = "/opt/axon/libaxon_pjrt.so"


def _ensure_ntff_hook():
    """concourse.bass_utils hard-imports antenv.axon_hooks when tracing is
    requested (BASS_TRACE=1). Some images lack that module; provide a shim
    wired to libaxon_pjrt.so's NRT profile entry points so tracing works
    (and a missing hook degrades to an untraced run instead of crashing)."""
    try:
        from antenv.axon_hooks import get_axon_ntff_profile_hook  # noqa: F401

        return
    except ImportError:
        pass

    import contextlib
    import ctypes
    import sys
    import types

    mod = types.ModuleType("antenv.axon_hooks")
    state = {"hook": None}
    mod.set_axon_ntff_profile_hook = lambda h: state.__setitem__("hook", h)
    mod.get_axon_ntff_profile_hook = lambda: state["hook"]
    sys.modules["antenv.axon_hooks"] = mod
    try:
        import antenv

        antenv.axon_hooks = mod
    except ImportError:
        pass

    if not os.path.exists(_AXON_PJRT_SO):
        return
    lib = ctypes.CDLL(_AXON_PJRT_SO)
    if not hasattr(lib, "axon_start_nrt_profile"):
        return
    lib.axon_start_nrt_profile.argtypes = [
        ctypes.POINTER(ctypes.c_int64),
        ctypes.c_size_t,
    ]
    lib.axon_start_nrt_profile.restype = ctypes.c_int64
    lib.axon_stop_nrt_profile.argtypes = [ctypes.c_char_p]
    lib.axon_stop_nrt_profile.restype = ctypes.c_int64

    @contextlib.contextmanager
    def _hook(output_dir, device_ids):
        import jax

        jax.devices()
        if device_ids:
            ids = (ctypes.c_int64 * len(device_ids))(*device_ids)
            rc = lib.axon_start_nrt_profile(ids, len(device_ids))
        else:
            rc = lib.axon_start_nrt_profile(None, 0)
        if rc != 0:
            raise RuntimeError(f"axon_start_nrt_profile rc={rc}")
        try:
            yield
        finally:
            n = lib.axon_stop_nrt_profile(str(output_dir).encode())
            if n < 0:
                raise RuntimeError(f"axon_stop_nrt_profile rc={n}")

    mod.set_axon_ntff_profile_hook(_hook)


def _build_nc(K, M_loc, P, C):
    import concourse.bacc as bacc
    import concourse.mybir as mybir
    from concourse.tile import TileContext

    f32 = mybir.dt.float32
    bf16 = mybir.dt.bfloat16
    NCH = P // C
    assert P % C == 0 and C % 512 == 0
    MMW = C // 512  # matmuls per chunk per tensor (PSUM bank = 512 f32)

    nc = bacc.Bacc(enable_partition_id=False)
    iou = nc.declare_dram_parameter("iou", [M_loc, P], bf16, isOutput=False)
    so2 = nc.declare_dram_parameter("so2", [K, P], bf16, isOutput=False)
    eo2 = nc.declare_dram_parameter("eo2", [K, P], bf16, isOutput=False)
    repl = nc.declare_dram_parameter("repl", [K, M_loc], bf16, isOutput=False)
    ntgt = nc.declare_dram_parameter("ntgt", [M_loc, 2], f32, isOutput=False)
    out = nc.declare_dram_parameter("out", [M_loc, 2], f32, isOutput=True)

    with TileContext(nc) as tc:
        with (
            tc.tile_pool(name="singles", bufs=1) as singles,
            # one slot per chunk: iou DMAs are all emitted up-front, so slots
            # must never be recycled (recycling would need WAR deps on readers
            # that don't exist yet at emission time)
            tc.tile_pool(name="io", bufs=P // C) as io,
            tc.tile_pool(name="work", bufs=3) as work,
            tc.tile_pool(name="psum", bufs=2, space="PSUM") as psum,
        ):
            R_sb = singles.tile([K, M_loc], bf16)
            nc.sync.dma_start(out=R_sb, in_=repl[:, :])
            ntgt_sb = singles.tile([M_loc, 2], f32)
            nc.sync.dma_start(out=ntgt_sb, in_=ntgt[:, :])
            # source-offset rows stay resident (bf16 [K, P] = K partitions x 32KB).
            # Loaded as one tile PER CHUNK-GROUP so early matmuls don't wait on
            # the whole 1MB transfer (Tile deps are per-tile). DMA emission
            # order: piece 0 + the first iou chunks FIRST so the pipeline
            # fills immediately, remaining pieces next, rest of iou after.
            so_piece = C
            so2_sbs, eo2_sbs, iou_tiles = [], [], []

            def load_piece(pi):
                psl = slice(pi * so_piece, (pi + 1) * so_piece)
                s_t = singles.tile([K, so_piece], bf16, tag=f"so2_sb{pi}")
                nc.sync.dma_start(out=s_t, in_=so2[:, psl])
                so2_sbs.append(s_t)
                e_t = singles.tile([K, so_piece], bf16, tag=f"eo2_sb{pi}")
                nc.sync.dma_start(out=e_t, in_=eo2[:, psl])
                eo2_sbs.append(e_t)

            def load_iou(ci):
                sl = slice(ci * C, (ci + 1) * C)
                t = io.tile([M_loc, C], bf16, tag="iouc")
                nc.sync.dma_start(out=t, in_=iou[:, sl])
                iou_tiles.append(t)

            # interleave so the first chunk's operands land first
            for ci in range(NCH):
                load_piece(ci)
                load_iou(ci)

            accL = singles.tile([M_loc, 2 * NCH], f32)
            NCH_DVE = max(0, min(NCH, (6 * NCH) // 16))  # ~3 of 8 at C=2048  # count chunks on DVE
            NCH_ACT = NCH - NCH_DVE                      # count chunks on ACT (Sign)
            accM = singles.tile([M_loc, max(NCH_DVE, 1)], f32)
            accS = singles.tile([M_loc, max(NCH_ACT, 1)], f32)
            neg_half = singles.tile([M_loc, 1], f32)
            nc.vector.memset(neg_half, -IOU_THRESHOLD)
            # fixed throwaway output tiles: same-engine WAW ordering only,
            # so no cross-engine release semaphores per chunk
            junk_dve = singles.tile([M_loc, C], bf16, tag="junk_dve")
            junk_act = singles.tile([M_loc, C], bf16, tag="junk_act")

            for ci in range(NCH):
                iouc = iou_tiles[ci]

                so2rep = psum.tile([M_loc, C], f32, tag="ps_s")
                eo2rep = psum.tile([M_loc, C], f32, tag="ps_e")
                for mi in range(MMW):
                    psl = slice(mi * 512, (mi + 1) * 512)
                    nc.tensor.matmul(
                        so2rep[:, psl], lhsT=R_sb,
                        rhs=so2_sbs[ci][:, psl],
                        start=True, stop=True,
                    )
                for mi in range(MMW):
                    psl = slice(mi * 512, (mi + 1) * 512)
                    nc.tensor.matmul(
                        eo2rep[:, psl], lhsT=R_sb,
                        rhs=eo2_sbs[ci][:, psl],
                        start=True, stop=True,
                    )

                a = work.tile([M_loc, C], bf16, tag="a")
                nc.scalar.activation(
                    out=a,
                    in_=so2rep,
                    func=mybir.ActivationFunctionType.Abs,
                    bias=ntgt_sb[:, 0:1],
                    scale=1.0,
                )
                b = work.tile([M_loc, C], bf16, tag="b")
                nc.scalar.activation(
                    out=b,
                    in_=eo2rep,
                    func=mybir.ActivationFunctionType.Abs,
                    bias=ntgt_sb[:, 1:2],
                    scale=1.0,
                )

                # NOTE: offloading an op to GPSIMD is a net loss here - GpSimd
                # and DVE share SBUF ports (exclusive lock) and both engines
                # drop to half rate when streaming concurrently.
                nc.vector.scalar_tensor_tensor(
                    out=junk_dve,
                    in0=iouc,
                    scalar=IOU_THRESHOLD,
                    in1=a,
                    op0=mybir.AluOpType.is_gt,
                    op1=mybir.AluOpType.mult,
                    accum_out=accL[:, ci : ci + 1],
                )
                nc.vector.scalar_tensor_tensor(
                    out=junk_dve,
                    in0=iouc,
                    scalar=IOU_THRESHOLD,
                    in1=b,
                    op0=mybir.AluOpType.is_gt,
                    op1=mybir.AluOpType.mult,
                    accum_out=accL[:, NCH + ci : NCH + ci + 1],
                )
                if ci < NCH_DVE:
                    # mask count on DVE (accum_out reduce op is op1)
                    nc.vector.tensor_scalar(
                        out=junk_dve,
                        in0=iouc,
                        scalar1=IOU_THRESHOLD,
                        scalar2=None,
                        op0=mybir.AluOpType.is_gt,
                        op1=mybir.AluOpType.add,
                        accum_out=accM[:, ci : ci + 1],
                    )
                else:
                    # mask count on ACT: accum of Sign(iou-0.5). The host
                    # nudges bf16 iou off the exact 0.5 value in both
                    # directions, so sign is strictly +-1 and
                    # count = (accum + C) / 2 exactly.
                    nc.scalar.activation(
                        out=junk_act,
                        in_=iouc,
                        func=mybir.ActivationFunctionType.Sign,
                        bias=neg_half[:, 0:1],
                        scale=1.0,
                        accum_out=accS[:, ci - NCH_DVE : ci - NCH_DVE + 1],
                    )

            outsb = singles.tile([M_loc, 2], f32)
            nc.vector.reduce_sum(
                out=outsb[:, 0:1], in_=accL, axis=mybir.AxisListType.X
            )
            # count = sum(accM) + (sum(accS) + NCH_ACT*C)/2
            cnt_m = singles.tile([M_loc, 1], f32)
            if NCH_DVE > 0:
                nc.vector.reduce_sum(out=cnt_m, in_=accM, axis=mybir.AxisListType.X)
            else:
                nc.vector.memset(cnt_m, 0.0)
            cnt_s = singles.tile([M_loc, 1], f32)
            if NCH_ACT > 0:
                nc.vector.reduce_sum(out=cnt_s, in_=accS, axis=mybir.AxisListType.X)
            else:
                nc.vector.memset(cnt_s, 0.0)
            cnt_s2 = singles.tile([M_loc, 1], f32)
            nc.vector.tensor_scalar(
                out=cnt_s2,
                in0=cnt_s,
                scalar1=0.5,
                scalar2=float(NCH_ACT * C) / 2.0,
                op0=mybir.AluOpType.mult,
                op1=mybir.AluOpType.add,
            )
            nc.vector.tensor_tensor(
                out=outsb[:, 1:2], in0=cnt_m, in1=cnt_s2,
                op=mybir.AluOpType.add,
            )
            nc.sync.dma_start(out=out[:, :], in_=outsb)

    nc.compile()
    return nc


def _scatter_m2s(num_targets, S, M):
    """target index -> source video index, mirroring jnp.repeat(
    arange(S), num_targets, total_repeat_length=M)."""
    cum = np.cumsum(num_targets.astype(np.int64))
    idx = np.searchsorted(cum, np.arange(M), side="right")
    return np.clip(idx, 0, S - 1).astype(np.int64)


def _numpy_reference(start_offset, end_offset, tgt_moments, num_targets, iou2ds, mask2d):
    """Exact numpy replica of reference.py (topk fallback path)."""
    M, N, _ = iou2ds.shape
    S, P = start_offset.shape
    scatter = _scatter_m2s(num_targets, S, M)
    so = start_offset[scatter]
    eo = end_offset[scatter]
    r, c = np.nonzero(mask2d)
    if r.shape[0] < P:
        pad = P - r.shape[0]
        r = np.concatenate([r, np.zeros(pad, dtype=r.dtype)])
        c = np.concatenate([c, np.zeros(pad, dtype=c.dtype)])
    else:
        r, c = r[:P], c[:P]
    iou1 = iou2ds.reshape(M, N * N)[:, r * N + c]
    # top-k scatter mask + threshold mask
    topk_idx = np.argsort(-iou1, axis=1, kind="stable")[:, :TOPK]
    mask = np.zeros((M, P), dtype=np.float32)
    np.put_along_axis(mask, topk_idx, 1.0, axis=1)
    mask = np.where(iou1 > IOU_THRESHOLD, np.float32(1.0), mask)
    starts = (r.astype(np.float32) / N)[None, :]
    ends = ((c.astype(np.float32) + 1.0) / N)[None, :]
    sot = tgt_moments[:, 0:1] - starts
    eot = tgt_moments[:, 1:2] - ends
    loss = np.abs(so - sot) + np.abs(eo - eot)
    return np.float32((loss * mask).sum(dtype=np.float64) / mask.sum(dtype=np.float64))


def kernel(**inputs):
    global LAST_EXEC_TIME_NS, LAST_RESULTS
    _ensure_ntff_hook()
    import ml_dtypes

    from concourse.bass_utils import run_bass_kernel_spmd

    start_offset = np.asarray(inputs["start_offset"], dtype=np.float32)
    end_offset = np.asarray(inputs["end_offset"], dtype=np.float32)
    tgt_moments = np.asarray(inputs["tgt_moments"], dtype=np.float32)
    num_targets = np.asarray(inputs["num_targets"])
    iou2ds = np.asarray(inputs["iou2ds"], dtype=np.float32)
    mask2d = np.asarray(inputs["mask2d"])

    bf16 = ml_dtypes.bfloat16

    M, N, _ = iou2ds.shape
    S, P = start_offset.shape
    assert M % N_CORES == 0
    M_loc = M // N_CORES

    # proposal-grid constants from mask2d (row-major nonzero, padded like jnp)
    r, c = np.nonzero(mask2d)
    if r.shape[0] < P:
        pad = P - r.shape[0]
        r = np.concatenate([r, np.zeros(pad, dtype=r.dtype)])
        c = np.concatenate([c, np.zeros(pad, dtype=c.dtype)])
    else:
        r, c = r[:P], c[:P]
    starts = r.astype(np.float32) / np.float32(N)
    ends = (c.astype(np.float32) + np.float32(1.0)) / np.float32(N)

    # iou1ds = iou2ds[:, r, c]; identity reshape when mask2d is all ones
    flat_idx = r.astype(np.int64) * N + c.astype(np.int64)
    iou_flat = iou2ds.reshape(M, N * N)
    if not (flat_idx == np.arange(P)).all():
        iou_flat = np.ascontiguousarray(iou_flat[:, flat_idx])
    # bf16 halves the iou DMA bytes, but values that round exactly onto the
    # 0.5 threshold would corrupt the comparison. Nudge those one bf16 ulp
    # away from 0.5 in the direction of their f32 value; this makes
    # (iou_bf16 > 0.5) == (iou_f32 > 0.5) for every element AND leaves no
    # element exactly at 0.5, so the device's Sign(iou-0.5) count path is
    # strictly +-1 (exact counts).
    iou_bf16 = iou_flat.astype(bf16)
    on_thr = iou_bf16 == bf16(IOU_THRESHOLD)
    above = on_thr & (iou_flat > np.float32(IOU_THRESHOLD))
    below = on_thr & ~above
    if above.any():
        iou_bf16[above] = bf16(0.50390625)  # nextafter(0.5, up) in bf16
    if below.any():
        iou_bf16[below] = bf16(0.498046875)  # nextafter(0.5, down) in bf16

    # fold grid constants into the offsets: loss_a = |so2 - tgt_s|
    so2_full = (start_offset + starts[None, :]).astype(bf16)
    eo2_full = (end_offset + ends[None, :]).astype(bf16)

    # per-core source-row windows + replication matrices
    scatter = _scatter_m2s(num_targets, S, M)
    src_lo = np.empty(N_CORES, dtype=np.int64)
    n_src = np.empty(N_CORES, dtype=np.int64)
    for core in range(N_CORES):
        seg = scatter[core * M_loc : (core + 1) * M_loc]
        src_lo[core] = seg[0]
        n_src[core] = seg[-1] - seg[0] + 1
    K = int(n_src.max())

    in_maps = []
    for core in range(N_CORES):
        seg = scatter[core * M_loc : (core + 1) * M_loc]
        lo = int(src_lo[core])
        so2_c = np.zeros((K, P), dtype=bf16)
        eo2_c = np.zeros((K, P), dtype=bf16)
        hi = min(lo + K, S)
        so2_c[: hi - lo] = so2_full[lo:hi]
        eo2_c[: hi - lo] = eo2_full[lo:hi]
        repl = np.zeros((K, M_loc), dtype=bf16)
        repl[seg - lo, np.arange(M_loc)] = 1.0
        ntgt = np.ascontiguousarray(
            -tgt_moments[core * M_loc : (core + 1) * M_loc, :]
        ).astype(np.float32)
        in_maps.append(
            {
                "iou": np.ascontiguousarray(iou_bf16[core * M_loc : (core + 1) * M_loc]),
                "so2": so2_c,
                "eo2": eo2_c,
                "repl": repl,
                "ntgt": ntgt,
            }
        )

    cache_key = (K, M_loc, P)
    if cache_key not in _NC_CACHE:
        _NC_CACHE[cache_key] = _build_nc(K, M_loc, P, C=1024)
    nc = _NC_CACHE[cache_key]

    res = run_bass_kernel_spmd(nc, in_maps, list(range(N_CORES)))
    LAST_EXEC_TIME_NS = res.exec_time_ns
    LAST_RESULTS = res

    loss_sum = 0.0
    mask_sum = 0.0
    min_count = np.inf
    for core in range(N_CORES):
        part = res.results[core]["out"]  # [M_loc, 2]
        loss_sum += part[:, 0].sum(dtype=np.float64)
        mask_sum += part[:, 1].sum(dtype=np.float64)
        min_count = min(min_count, part[:, 1].min())

    if min_count < TOPK:
        # some row's top-k reaches below the threshold: the threshold mask is
        # not exact there -> use the exact (slow) host path
        return _numpy_reference(
            start_offset, end_offset, tgt_moments, num_targets, iou2ds, mask2d
        )

    return np.float32(loss_sum / mask_sum)


# revision 38
# speedup vs baseline: 1.0765x; 1.0279x over previous
"""Trainium2 Bass kernel for nn_BboxRegressionLoss (topk_masking).

Math notes
----------
reference computes, with iou1ds = iou2ds reshaped [M, P] (mask2d all-ones):
    mask = scatter(top3_idx) | (iou1ds > 0.5)
    loss = |so + starts - tgt_s| + |eo + ends - tgt_e|     (per [M, P] element)
    out  = (loss * mask).sum() / mask.sum()

Key identity: if a row has >= TOPK elements with iou > 0.5, its top-TOPK
elements are all already inside the threshold mask, so mask == (iou > 0.5)
EXACTLY for that row. We compute per-row counts of (iou > 0.5) on device
anyway (needed for mask.sum()), so we can verify the identity for every row
after the fact and fall back to a full numpy replica in the (practically
impossible for uniform iou) case where some row has fewer than TOPK
above-threshold elements.

Device layout (per core, M_loc = 128 targets on partitions, P chunked):
    PE     : replicate K source-offset rows -> 128 target partitions via a
             0/1 matmul (avoids re-reading so/eo 4x from HBM)
    ACT    : a = Abs(so2rep - tgt_s), b = Abs(eo2rep - tgt_e)   (bias fusion)
    DVE    : scalar_tensor_tensor (iou > 0.5) * a  with fused row-sum accum
             (and same for b); tensor_scalar (iou > 0.5) accum for the count
Host folds the `starts`/`ends` proposal-grid constants into so/eo (so2/eo2),
sums the 8 x [128, 2] partials in f64 and divides.

bf16 is used for iou/so2/eo2/a/b (DVE 16-bit 2x mode + half the DMA bytes);
accumulation stays f32. Measured end-to-end rel err vs the f32 reference is
~1e-4, far inside the tolerance.
"""

import os

import numpy as np

TOPK = 3
IOU_THRESHOLD = 0.5
N_CORES = 8

# filled by kernel() on every call; test.py reads these
LAST_EXEC_TIME_NS = None
LAST_RESULTS = None

_NC_CACHE = {}

_AXON_PJRT_SO = "/opt/axon/libaxon_pjrt.so"


def _ensure_ntff_hook():
    """concourse.bass_utils hard-imports antenv.axon_hooks when tracing is
    requested (BASS_TRACE=1). Some images lack that module; provide a shim
    wired to libaxon_pjrt.so's NRT profile entry points so tracing works
    (and a missing hook degrades to an untraced run instead of crashing)."""
    try:
        from antenv.axon_hooks import get_axon_ntff_profile_hook  # noqa: F401

        return
    except ImportError:
        pass

    import contextlib
    import ctypes
    import sys
    import types

    mod = types.ModuleType("antenv.axon_hooks")
    state = {"hook": None}
    mod.set_axon_ntff_profile_hook = lambda h: state.__setitem__("hook", h)
    mod.get_axon_ntff_profile_hook = lambda: state["hook"]
    sys.modules["antenv.axon_hooks"] = mod
    try:
        import antenv

        antenv.axon_hooks = mod
    except ImportError:
        pass

    if not os.path.exists(_AXON_PJRT_SO):
        return
    lib = ctypes.CDLL(_AXON_PJRT_SO)
    if not hasattr(lib, "axon_start_nrt_profile"):
        return
    lib.axon_start_nrt_profile.argtypes = [
        ctypes.POINTER(ctypes.c_int64),
        ctypes.c_size_t,
    ]
    lib.axon_start_nrt_profile.restype = ctypes.c_int64
    lib.axon_stop_nrt_profile.argtypes = [ctypes.c_char_p]
    lib.axon_stop_nrt_profile.restype = ctypes.c_int64

    @contextlib.contextmanager
    def _hook(output_dir, device_ids):
        import jax

        jax.devices()
        if device_ids:
            ids = (ctypes.c_int64 * len(device_ids))(*device_ids)
            rc = lib.axon_start_nrt_profile(ids, len(device_ids))
        else:
            rc = lib.axon_start_nrt_profile(None, 0)
        if rc != 0:
            raise RuntimeError(f"axon_start_nrt_profile rc={rc}")
        try:
            yield
        finally:
            n = lib.axon_stop_nrt_profile(str(output_dir).encode())
            if n < 0:
                raise RuntimeError(f"axon_stop_nrt_profile rc={n}")

    mod.set_axon_ntff_profile_hook(_hook)


def _build_nc(K, M_loc, P, C):
    import concourse.bacc as bacc
    import concourse.bass as bass
    import concourse.mybir as mybir
    from concourse.tile import TileContext

    f32 = mybir.dt.float32
    bf16 = mybir.dt.bfloat16
    NCH = P // C
    assert P % C == 0 and C % 512 == 0
    MMW = C // 512  # matmuls per chunk per tensor (PSUM bank = 512 f32)

    nc = bacc.Bacc(enable_partition_id=False)
    iou = nc.declare_dram_parameter("iou", [M_loc, P], bf16, isOutput=False)
    so2 = nc.declare_dram_parameter("so2", [K, P], bf16, isOutput=False)
    eo2 = nc.declare_dram_parameter("eo2", [K, P], bf16, isOutput=False)
    repl = nc.declare_dram_parameter("repl", [K, M_loc], bf16, isOutput=False)
    ntgt = nc.declare_dram_parameter("ntgt", [M_loc, 2], f32, isOutput=False)
    out = nc.declare_dram_parameter("out", [M_loc, 2], f32, isOutput=True)

    with TileContext(nc) as tc:
        with (
            tc.tile_pool(name="singles", bufs=1) as singles,
            # one slot per chunk: iou DMAs are all emitted up-front, so slots
            # must never be recycled (recycling would need WAR deps on readers
            # that don't exist yet at emission time)
            tc.tile_pool(name="io", bufs=P // C) as io,
            tc.tile_pool(name="work", bufs=3) as work,
            tc.tile_pool(name="psum", bufs=2, space="PSUM") as psum,
        ):
            R_sb = singles.tile([K, M_loc], bf16)
            nc.sync.dma_start(out=R_sb, in_=repl[:, :])
            ntgt_sb = singles.tile([M_loc, 2], f32)
            nc.sync.dma_start(out=ntgt_sb, in_=ntgt[:, :])
            # source-offset rows stay resident (bf16 [K, P] = K partitions x 32KB).
            # Loaded as one tile PER CHUNK-GROUP so early matmuls don't wait on
            # the whole 1MB transfer (Tile deps are per-tile). DMA emission
            # order: piece 0 + the first iou chunks FIRST so the pipeline
            # fills immediately, remaining pieces next, rest of iou after.
            so_piece = C
            so2_sbs, eo2_sbs, iou_tiles = [], [], []

            def load_piece(pi):
                psl = slice(pi * so_piece, (pi + 1) * so_piece)
                s_t = singles.tile([K, so_piece], bf16, tag=f"so2_sb{pi}")
                nc.sync.dma_start(out=s_t, in_=so2[:, psl])
                so2_sbs.append(s_t)
                e_t = singles.tile([K, so_piece], bf16, tag=f"eo2_sb{pi}")
                nc.sync.dma_start(out=e_t, in_=eo2[:, psl])
                eo2_sbs.append(e_t)

            def load_iou(ci):
                sl = slice(ci * C, (ci + 1) * C)
                t = io.tile([M_loc, C], bf16, tag="iouc")
                nc.sync.dma_start(out=t, in_=iou[:, sl])
                iou_tiles.append(t)

            # interleave so the first chunk's operands land first
            for ci in range(NCH):
                load_piece(ci)
                load_iou(ci)

            accL = singles.tile([M_loc, 2 * NCH], f32)
            NCH_DVE = max(0, min(NCH, (6 * NCH) // 16))  # ~3 of 8 at C=2048  # count chunks on DVE
            NCH_ACT = NCH - NCH_DVE                      # count chunks on ACT (Sign)
            accM = singles.tile([M_loc, max(NCH_DVE, 1)], f32)
            accS = singles.tile([M_loc, max(NCH_ACT, 1)], f32)
            neg_half = singles.tile([M_loc, 1], f32)
            nc.vector.memset(neg_half, -IOU_THRESHOLD)
            # fixed throwaway output tiles: same-engine WAW ordering only,
            # so no cross-engine release semaphores per chunk
            junk_dve = singles.tile([M_loc, 2 * C], bf16, tag="junk_dve")
            junk_act = singles.tile([M_loc, C], bf16, tag="junk_act")

            for ci in range(NCH):
                iouc = iou_tiles[ci]

                so2rep = psum.tile([M_loc, C], f32, tag="ps_s")
                eo2rep = psum.tile([M_loc, C], f32, tag="ps_e")
                for mi in range(MMW):
                    psl = slice(mi * 512, (mi + 1) * 512)
                    nc.tensor.matmul(
                        so2rep[:, psl], lhsT=R_sb,
                        rhs=so2_sbs[ci][:, psl],
                        start=True, stop=True,
                    )
                for mi in range(MMW):
                    psl = slice(mi * 512, (mi + 1) * 512)
                    nc.tensor.matmul(
                        eo2rep[:, psl], lhsT=R_sb,
                        rhs=eo2_sbs[ci][:, psl],
                        start=True, stop=True,
                    )

                ab = work.tile([M_loc, 2, C], bf16, tag="ab")
                nc.scalar.activation(
                    out=ab[:, 0, :],
                    in_=so2rep,
                    func=mybir.ActivationFunctionType.Abs,
                    bias=ntgt_sb[:, 0:1],
                    scale=1.0,
                )
                nc.scalar.activation(
                    out=ab[:, 1, :],
                    in_=eo2rep,
                    func=mybir.ActivationFunctionType.Abs,
                    bias=ntgt_sb[:, 1:2],
                    scale=1.0,
                )

                # NOTE: offloading an op to GPSIMD is a net loss here - GpSimd
                # and DVE share SBUF ports (exclusive lock) and both engines
                # drop to half rate when streaming concurrently.
                nc.vector.scalar_tensor_tensor(
                    out=junk_dve[:, 0:C],
                    in0=iouc,
                    scalar=IOU_THRESHOLD,
                    in1=ab[:, 0, :],
                    op0=mybir.AluOpType.is_gt,
                    op1=mybir.AluOpType.mult,
                    accum_out=accL[:, ci : ci + 1],
                )
                nc.vector.scalar_tensor_tensor(
                    out=junk_dve[:, C : 2 * C],
                    in0=iouc,
                    scalar=IOU_THRESHOLD,
                    in1=ab[:, 1, :],
                    op0=mybir.AluOpType.is_gt,
                    op1=mybir.AluOpType.mult,
                    accum_out=accL[:, NCH + ci : NCH + ci + 1],
                )
                if ci < NCH_DVE:
                    # mask count on DVE (accum_out reduce op is op1)
                    nc.vector.tensor_scalar(
                        out=junk_dve[:, 0:C],
                        in0=iouc,
                        scalar1=IOU_THRESHOLD,
                        scalar2=None,
                        op0=mybir.AluOpType.is_gt,
                        op1=mybir.AluOpType.add,
                        accum_out=accM[:, ci : ci + 1],
                    )
                else:
                    # mask count on ACT: accum of Sign(iou-0.5). The host
                    # nudges bf16 iou off the exact 0.5 value in both
                    # directions, so sign is strictly +-1 and
                    # count = (accum + C) / 2 exactly.
                    nc.scalar.activation(
                        out=junk_act[:, 0:C],
                        in_=iouc,
                        func=mybir.ActivationFunctionType.Sign,
                        bias=neg_half[:, 0:1],
                        scale=1.0,
                        accum_out=accS[:, ci - NCH_DVE : ci - NCH_DVE + 1],
                    )

            outsb = singles.tile([M_loc, 2], f32)
            nc.vector.reduce_sum(
                out=outsb[:, 0:1], in_=accL, axis=mybir.AxisListType.X
            )
            # count = sum(accM) + (sum(accS) + NCH_ACT*C)/2
            cnt_m = singles.tile([M_loc, 1], f32)
            if NCH_DVE > 0:
                nc.vector.reduce_sum(out=cnt_m, in_=accM, axis=mybir.AxisListType.X)
            else:
                nc.vector.memset(cnt_m, 0.0)
            cnt_s = singles.tile([M_loc, 1], f32)
            if NCH_ACT > 0:
                nc.vector.reduce_sum(out=cnt_s, in_=accS, axis=mybir.AxisListType.X)
            else:
                nc.vector.memset(cnt_s, 0.0)
            cnt_s2 = singles.tile([M_loc, 1], f32)
            nc.vector.tensor_scalar(
                out=cnt_s2,
                in0=cnt_s,
                scalar1=0.5,
                scalar2=float(NCH_ACT * C) / 2.0,
                op0=mybir.AluOpType.mult,
                op1=mybir.AluOpType.add,
            )
            nc.vector.tensor_tensor(
                out=outsb[:, 1:2], in0=cnt_m, in1=cnt_s2,
                op=mybir.AluOpType.add,
            )
            nc.sync.dma_start(out=out[:, :], in_=outsb)

    nc.compile()
    return nc


def _scatter_m2s(num_targets, S, M):
    """target index -> source video index, mirroring jnp.repeat(
    arange(S), num_targets, total_repeat_length=M)."""
    cum = np.cumsum(num_targets.astype(np.int64))
    idx = np.searchsorted(cum, np.arange(M), side="right")
    return np.clip(idx, 0, S - 1).astype(np.int64)


def _numpy_reference(start_offset, end_offset, tgt_moments, num_targets, iou2ds, mask2d):
    """Exact numpy replica of reference.py (topk fallback path)."""
    M, N, _ = iou2ds.shape
    S, P = start_offset.shape
    scatter = _scatter_m2s(num_targets, S, M)
    so = start_offset[scatter]
    eo = end_offset[scatter]
    r, c = np.nonzero(mask2d)
    if r.shape[0] < P:
        pad = P - r.shape[0]
        r = np.concatenate([r, np.zeros(pad, dtype=r.dtype)])
        c = np.concatenate([c, np.zeros(pad, dtype=c.dtype)])
    else:
        r, c = r[:P], c[:P]
    iou1 = iou2ds.reshape(M, N * N)[:, r * N + c]
    # top-k scatter mask + threshold mask
    topk_idx = np.argsort(-iou1, axis=1, kind="stable")[:, :TOPK]
    mask = np.zeros((M, P), dtype=np.float32)
    np.put_along_axis(mask, topk_idx, 1.0, axis=1)
    mask = np.where(iou1 > IOU_THRESHOLD, np.float32(1.0), mask)
    starts = (r.astype(np.float32) / N)[None, :]
    ends = ((c.astype(np.float32) + 1.0) / N)[None, :]
    sot = tgt_moments[:, 0:1] - starts
    eot = tgt_moments[:, 1:2] - ends
    loss = np.abs(so - sot) + np.abs(eo - eot)
    return np.float32((loss * mask).sum(dtype=np.float64) / mask.sum(dtype=np.float64))


def kernel(**inputs):
    global LAST_EXEC_TIME_NS, LAST_RESULTS
    _ensure_ntff_hook()
    import ml_dtypes

    from concourse.bass_utils import run_bass_kernel_spmd

    start_offset = np.asarray(inputs["start_offset"], dtype=np.float32)
    end_offset = np.asarray(inputs["end_offset"], dtype=np.float32)
    tgt_moments = np.asarray(inputs["tgt_moments"], dtype=np.float32)
    num_targets = np.asarray(inputs["num_targets"])
    iou2ds = np.asarray(inputs["iou2ds"], dtype=np.float32)
    mask2d = np.asarray(inputs["mask2d"])

    bf16 = ml_dtypes.bfloat16

    M, N, _ = iou2ds.shape
    S, P = start_offset.shape
    assert M % N_CORES == 0
    M_loc = M // N_CORES

    # proposal-grid constants from mask2d (row-major nonzero, padded like jnp)
    r, c = np.nonzero(mask2d)
    if r.shape[0] < P:
        pad = P - r.shape[0]
        r = np.concatenate([r, np.zeros(pad, dtype=r.dtype)])
        c = np.concatenate([c, np.zeros(pad, dtype=c.dtype)])
    else:
        r, c = r[:P], c[:P]
    starts = r.astype(np.float32) / np.float32(N)
    ends = (c.astype(np.float32) + np.float32(1.0)) / np.float32(N)

    # iou1ds = iou2ds[:, r, c]; identity reshape when mask2d is all ones
    flat_idx = r.astype(np.int64) * N + c.astype(np.int64)
    iou_flat = iou2ds.reshape(M, N * N)
    if not (flat_idx == np.arange(P)).all():
        iou_flat = np.ascontiguousarray(iou_flat[:, flat_idx])
    # bf16 halves the iou DMA bytes, but values that round exactly onto the
    # 0.5 threshold would corrupt the comparison. Nudge those one bf16 ulp
    # away from 0.5 in the direction of their f32 value; this makes
    # (iou_bf16 > 0.5) == (iou_f32 > 0.5) for every element AND leaves no
    # element exactly at 0.5, so the device's Sign(iou-0.5) count path is
    # strictly +-1 (exact counts).
    iou_bf16 = iou_flat.astype(bf16)
    on_thr = iou_bf16 == bf16(IOU_THRESHOLD)
    above = on_thr & (iou_flat > np.float32(IOU_THRESHOLD))
    below = on_thr & ~above
    if above.any():
        iou_bf16[above] = bf16(0.50390625)  # nextafter(0.5, up) in bf16
    if below.any():
        iou_bf16[below] = bf16(0.498046875)  # nextafter(0.5, down) in bf16

    # fold grid constants into the offsets: loss_a = |so2 - tgt_s|
    so2_full = (start_offset + starts[None, :]).astype(bf16)
    eo2_full = (end_offset + ends[None, :]).astype(bf16)

    # per-core source-row windows + replication matrices
    scatter = _scatter_m2s(num_targets, S, M)
    src_lo = np.empty(N_CORES, dtype=np.int64)
    n_src = np.empty(N_CORES, dtype=np.int64)
    for core in range(N_CORES):
        seg = scatter[core * M_loc : (core + 1) * M_loc]
        src_lo[core] = seg[0]
        n_src[core] = seg[-1] - seg[0] + 1
    K = int(n_src.max())

    in_maps = []
    for core in range(N_CORES):
        seg = scatter[core * M_loc : (core + 1) * M_loc]
        lo = int(src_lo[core])
        so2_c = np.zeros((K, P), dtype=bf16)
        eo2_c = np.zeros((K, P), dtype=bf16)
        hi = min(lo + K, S)
        so2_c[: hi - lo] = so2_full[lo:hi]
        eo2_c[: hi - lo] = eo2_full[lo:hi]
        repl = np.zeros((K, M_loc), dtype=bf16)
        repl[seg - lo, np.arange(M_loc)] = 1.0
        ntgt = np.ascontiguousarray(
            -tgt_moments[core * M_loc : (core + 1) * M_loc, :]
        ).astype(np.float32)
        in_maps.append(
            {
                "iou": np.ascontiguousarray(iou_bf16[core * M_loc : (core + 1) * M_loc]),
                "so2": so2_c,
                "eo2": eo2_c,
                "repl": repl,
                "ntgt": ntgt,
            }
        )

    cache_key = (K, M_loc, P)
    if cache_key not in _NC_CACHE:
        _NC_CACHE[cache_key] = _build_nc(K, M_loc, P, C=1024)
    nc = _NC_CACHE[cache_key]

    res = run_bass_kernel_spmd(nc, in_maps, list(range(N_CORES)))
    LAST_EXEC_TIME_NS = res.exec_time_ns
    LAST_RESULTS = res

    loss_sum = 0.0
    mask_sum = 0.0
    min_count = np.inf
    for core in range(N_CORES):
        part = res.results[core]["out"]  # [M_loc, 2]
        loss_sum += part[:, 0].sum(dtype=np.float64)
        mask_sum += part[:, 1].sum(dtype=np.float64)
        min_count = min(min_count, part[:, 1].min())

    if min_count < TOPK:
        # some row's top-k reaches below the threshold: the threshold mask is
        # not exact there -> use the exact (slow) host path
        return _numpy_reference(
            start_offset, end_offset, tgt_moments, num_targets, iou2ds, mask2d
        )

    return np.float32(loss_sum / mask_sum)
